# revision 1
# baseline (speedup 1.0000x reference)
"""Supervised-contrastive loss (nn_ConLoss) on 8 Trainium2 NeuronCores.

Strategy (per sharding hint): shard feature rows across the 8 cores
(1024 rows each). Every core holds the full transposed feature matrix in
SBUF — column-permuted so its own 1024 rows sit at columns 0..1023, which
keeps one SPMD program with static APs — computes its [1024, 8192] block
of Z = X X^T / T on the tensor engine (fp32r), does the diagonal-masked
row max / sum-exp / label-mask sums locally, and emits one per-row loss
vector. The host sums the 8 partial vectors and divides by sum(weights).
"""
import numpy as np

TEMPERATURE = 0.1
N, D, C = 8192, 512, 8
R = N // C            # 1024 rows per core
NRB = R // 128        # 8 row blocks of 128
CW = 1024             # elementwise tile width (2 psum banks)
NCC = N // CW         # 8 column chunks

_NC_CACHE = {}


def _build_nc():
    if "nc" in _NC_CACHE:
        return _NC_CACHE["nc"]
    import concourse.tile as tile
    from concourse import bacc, mybir
    from contextlib import ExitStack

    DT = mybir.dt
    ALU = mybir.AluOpType
    ACTF = mybir.ActivationFunctionType

    nc = bacc.Bacc("TRN2", target_bir_lowering=False, debug=False)
    xt_d = nc.dram_tensor("xt", [D, N], DT.float32r, kind="ExternalInput")
    lcol_d = nc.dram_tensor("labcol", [128, N], DT.bfloat16, kind="ExternalInput")
    lrow_d = nc.dram_tensor("labrow", [128, NRB], DT.float32, kind="ExternalInput")
    negw_d = nc.dram_tensor("negw", [128, NRB], DT.float32, kind="ExternalInput")
    icnt_d = nc.dram_tensor("icnt10", [128, NRB], DT.float32, kind="ExternalInput")
    ome_d = nc.dram_tensor("ome", [128, 128], DT.float32, kind="ExternalInput")
    res_d = nc.dram_tensor("res", [NRB, 128], DT.float32, kind="ExternalOutput")

    with tile.TileContext(nc) as tc, ExitStack() as ctx:
        xt_pool = ctx.enter_context(tc.tile_pool(name="xt", bufs=1))
        lab_pool = ctx.enter_context(tc.tile_pool(name="lab", bufs=1))
        small_pool = ctx.enter_context(tc.tile_pool(name="small", bufs=1))
        z_pool = ctx.enter_context(tc.tile_pool(name="z", bufs=1))
        ps_pool = ctx.enter_context(tc.tile_pool(name="ps", bufs=4, space="PSUM"))
        scr_pool = ctx.enter_context(tc.tile_pool(name="scr", bufs=2))
        st_pool = ctx.enter_context(tc.tile_pool(name="st", bufs=3))

        xt_sb = {}
        for cc in range(NCC):
            for k in range(4):
                t = xt_pool.tile([128, CW], DT.float32r, tag=f"xt_{k}_{cc}")
                nc.sync.dma_start(t[:], xt_d[k * 128:(k + 1) * 128, cc * CW:(cc + 1) * CW])
                xt_sb[k, cc] = t
        lcol_sb = []
        for cc in range(NCC):
            t = lab_pool.tile([128, CW], DT.bfloat16, tag=f"lab_{cc}")
            nc.sync.dma_start(t[:], lcol_d[:, cc * CW:(cc + 1) * CW])
            lcol_sb.append(t)
        lrow_sb = small_pool.tile([128, NRB], DT.float32)
        nc.sync.dma_start(lrow_sb[:], lrow_d[:])
        negw_sb = small_pool.tile([128, NRB], DT.float32)
        nc.sync.dma_start(negw_sb[:], negw_d[:])
        icnt_sb = small_pool.tile([128, NRB], DT.float32)
        nc.sync.dma_start(icnt_sb[:], icnt_d[:])
        ome_sb = small_pool.tile([128, 128], DT.float32)
        nc.sync.dma_start(ome_sb[:], ome_d[:])

        for rb in range(NRB):
            rm = st_pool.tile([128, NCC], DT.float32, tag="rm")
            sp = st_pool.tile([128, NCC], DT.float32, tag="sp")
            zp = st_pool.tile([128, NCC], DT.float32, tag="zp")

            z_tiles = []
            for cc in range(NCC):
                ps = ps_pool.tile([128, CW], DT.float32, tag="ps")
                for h in range(2):
                    for k in range(4):
                        nc.tensor.matmul(
                            ps[:, h * 512:(h + 1) * 512],
                            xt_sb[k, 0][:, rb * 128:(rb + 1) * 128],
                            xt_sb[k, cc][:, h * 512:(h + 1) * 512],
                            start=(k == 0), stop=(k == 3))
                if cc == 0:
                    off = rb * 128
                    nc.vector.scalar_tensor_tensor(
                        out=ps[:, off:off + 128], in0=ps[:, off:off + 128],
                        scalar=0.0, in1=ome_sb[:],
                        op0=ALU.bypass, op1=ALU.mult)
                zt = z_pool.tile([128, CW], DT.float32, tag=f"z_{cc}")
                # fused PSUM->SBUF copy + per-tile row max
                nc.vector.tensor_scalar(
                    out=zt[:], in0=ps[:], scalar1=0.0, scalar2=-3.0e38,
                    op0=ALU.add, op1=ALU.max, accum_out=rm[:, cc:cc + 1])
                z_tiles.append(zt)

            mfin = st_pool.tile([128, 1], DT.float32, tag="mfin")
            nc.vector.tensor_reduce(mfin[:], rm[:], axis=mybir.AxisListType.X,
                                    op=ALU.max)
            negm = st_pool.tile([128, 1], DT.float32, tag="negm")
            nc.vector.tensor_scalar_mul(negm[:], mfin[:], -10.0)

            for cc in range(NCC):
                mscr = scr_pool.tile([128, CW], DT.bfloat16, tag="mscr")
                nc.vector.scalar_tensor_tensor(
                    out=mscr[:], in0=lcol_sb[cc][:], scalar=lrow_sb[:, rb:rb + 1],
                    in1=z_tiles[cc][:], op0=ALU.is_equal, op1=ALU.mult,
                    accum_out=zp[:, cc:cc + 1])
                escr = scr_pool.tile([128, CW], DT.bfloat16, tag="escr")
                nc.scalar.activation(
                    out=escr[:], in_=z_tiles[cc][:], func=ACTF.Exp,
                    bias=negm[:], scale=10.0, accum_out=sp[:, cc:cc + 1])

            ssum = st_pool.tile([128, 1], DT.float32, tag="ssum")
            nc.vector.reduce_sum(ssum[:], sp[:], axis=mybir.AxisListType.X)
            lns = st_pool.tile([128, 1], DT.float32, tag="lns")
            nc.scalar.activation(lns[:], ssum[:], ACTF.Ln)
            lse = st_pool.tile([128, 1], DT.float32, tag="lse")
            nc.vector.tensor_sub(lse[:], lns[:], negm[:])
            zsum = st_pool.tile([128, 1], DT.float32, tag="zsum")
            nc.vector.reduce_sum(zsum[:], zp[:], axis=mybir.AxisListType.X)
            tmp = st_pool.tile([128, 1], DT.float32, tag="tmp")
            nc.vector.scalar_tensor_tensor(
                out=tmp[:], in0=zsum[:], scalar=icnt_sb[:, rb:rb + 1], in1=lse[:],
                op0=ALU.mult, op1=ALU.subtract)
            resv = st_pool.tile([128, 1], DT.float32, tag="resv")
            nc.vector.tensor_scalar(
                out=resv[:], in0=tmp[:], scalar1=negw_sb[:, rb:rb + 1],
                scalar2=None, op0=ALU.mult)
            nc.sync.dma_start(res_d[rb, :], resv[:])

    nc.compile()
    _NC_CACHE["nc"] = nc
    return nc


def _reset_device():
    try:
        import ctypes, jax
        jax.devices()
        ctypes.CDLL("/opt/axon/libaxon_pjrt.so").axon_reset()
    except Exception:
        pass


def _make_in_maps(features, labels, weights):
    import ml_dtypes

    f = np.ascontiguousarray(np.asarray(features, dtype=np.float32))
    lab = np.asarray(labels).astype(np.int32)
    w = np.asarray(weights, dtype=np.float32)

    xt = np.ascontiguousarray(f.T)                      # [D, N]
    lab_bf = lab.astype(ml_dtypes.bfloat16)
    ome = (1.0 - np.eye(128)).astype(np.float32)
    hist = np.bincount(lab, minlength=100).astype(np.float64)
    icnt10_full = (10.0 / (hist[lab] - 1.0)).astype(np.float32)   # [N]

    in_maps = []
    for c in range(C):
        sl = slice(c * R, (c + 1) * R)
        perm = np.concatenate([
            np.arange(c * R, (c + 1) * R),
            np.arange(0, c * R),
            np.arange((c + 1) * R, N),
        ])
        in_maps.append({
            "xt": np.ascontiguousarray(xt[:, perm]),
            "labcol": np.ascontiguousarray(
                np.broadcast_to(lab_bf[perm][None, :], (128, N))),
            "labrow": np.ascontiguousarray(
                lab[sl].astype(np.float32).reshape(NRB, 128).T),
            "negw": np.ascontiguousarray(
                (-w[sl]).reshape(NRB, 128).T),
            "icnt10": np.ascontiguousarray(
                icnt10_full[sl].reshape(NRB, 128).T),
            "ome": ome,
        })

    return in_maps


def kernel(features, labels, weights):
    from concourse.bass_utils import run_bass_kernel_spmd

    w = np.asarray(weights, dtype=np.float32)
    nc = _build_nc()
    _reset_device()
    in_maps = _make_in_maps(features, labels, weights)
    out = run_bass_kernel_spmd(nc, in_maps, list(range(C)))
    total = np.float64(0.0)
    for c in range(C):
        total += out.results[c]["res"].astype(np.float64).sum()
    loss = total / np.float64(w.astype(np.float64).sum())
    return np.asarray(loss, dtype=np.float32)



# revision 2
# speedup vs baseline: 1.5346x; 1.5346x over previous
"""Supervised-contrastive loss (nn_ConLoss) on 8 Trainium2 NeuronCores.

Strategy: shard feature rows across the 8 cores (1024 rows each). Every
core holds the full transposed feature matrix in SBUF in fp16 — column-
permuted so its own 1024 rows sit at columns 0..1023 — computes its
[1024, 8192] block of Z = X X^T on the tensor engine in fp16 (2x the
fp32r rate), does the diagonal-masked row max / sum-exp locally, and
computes the positive-pair sums algebraically: sum_{j: lab_j=lab_i} z_ij
= x_i . S_{lab_i} - ||x_i||^2 where S = per-class feature sums, via a
tiny [1024,100] matmul + one-hot dot — no O(N^2) mask pass needed.
The host sums the 8 partial row-loss vectors and divides by sum(weights).
"""
import numpy as np

TEMPERATURE = 0.1
N, D, C = 8192, 512, 8
R = N // C            # 1024 rows per core
NRB = R // 128        # 8 row blocks of 128
CW = 1024             # elementwise tile width (2 psum banks)
NCC = N // CW         # 8 column chunks
NK = D // 128         # 4 k-tiles
NL = 100              # num classes

_NC_CACHE = {}


def _build_nc():
    if "nc" in _NC_CACHE:
        return _NC_CACHE["nc"]
    import concourse.tile as tile
    from concourse import bacc, mybir
    from contextlib import ExitStack

    DT = mybir.dt
    ALU = mybir.AluOpType
    ACTF = mybir.ActivationFunctionType

    nc = bacc.Bacc("TRN2", target_bir_lowering=False, debug=False)
    xt_d = nc.dram_tensor("xt", [D, N], DT.float16, kind="ExternalInput")
    st_d = nc.dram_tensor("st", [D, NL], DT.float16, kind="ExternalInput")
    lrow_d = nc.dram_tensor("labrow", [128, NRB], DT.float32, kind="ExternalInput")
    negw_d = nc.dram_tensor("negw", [128, NRB], DT.float32, kind="ExternalInput")
    icnt_d = nc.dram_tensor("icnt10", [128, NRB], DT.float32, kind="ExternalInput")
    nrm_d = nc.dram_tensor("nrm", [128, NRB], DT.float32, kind="ExternalInput")
    kidx_d = nc.dram_tensor("kidx", [128, NL], DT.float32, kind="ExternalInput")
    ome_d = nc.dram_tensor("ome", [128, 128], DT.float32, kind="ExternalInput")
    res_d = nc.dram_tensor("res", [128, NRB], DT.float32, kind="ExternalOutput")

    with tile.TileContext(nc) as tc, ExitStack() as ctx:
        xt_pool = ctx.enter_context(tc.tile_pool(name="xt", bufs=1))
        sml_pool = ctx.enter_context(tc.tile_pool(name="sml", bufs=1))
        z_pool = ctx.enter_context(tc.tile_pool(name="z", bufs=2))
        e_pool = ctx.enter_context(tc.tile_pool(name="e", bufs=2))
        ps_pool = ctx.enter_context(tc.tile_pool(name="ps", bufs=3, space="PSUM"))
        g_pool = ctx.enter_context(tc.tile_pool(name="g", bufs=2, space="PSUM"))
        st_pool = ctx.enter_context(tc.tile_pool(name="st", bufs=2))
        acc_pool = ctx.enter_context(tc.tile_pool(name="acc", bufs=1))

        # small inputs first (G matmuls for rb=0 need st tiles early)
        st_sb = []
        for k in range(NK):
            t = sml_pool.tile([128, NL], DT.float16, tag=f"st_{k}")
            nc.sync.dma_start(t[:], st_d[k * 128:(k + 1) * 128, :])
            st_sb.append(t)
        lrow_sb = sml_pool.tile([128, NRB], DT.float32)
        nc.sync.dma_start(lrow_sb[:], lrow_d[:])
        negw_sb = sml_pool.tile([128, NRB], DT.float32)
        nc.sync.dma_start(negw_sb[:], negw_d[:])
        icnt_sb = sml_pool.tile([128, NRB], DT.float32)
        nc.sync.dma_start(icnt_sb[:], icnt_d[:])
        nrm_sb = sml_pool.tile([128, NRB], DT.float32)
        nc.sync.dma_start(nrm_sb[:], nrm_d[:])
        kidx_sb = sml_pool.tile([128, NL], DT.float32)
        nc.sync.dma_start(kidx_sb[:], kidx_d[:])
        ome_sb = sml_pool.tile([128, 128], DT.float32)
        nc.sync.dma_start(ome_sb[:], ome_d[:])

        # xt tiles, chunk-major so chunk 0 (the stationary side) lands first
        xt_sb = {}
        for cc in range(NCC):
            for k in range(NK):
                t = xt_pool.tile([128, CW], DT.float16, tag=f"xt_{k}_{cc}")
                nc.sync.dma_start(t[:], xt_d[k * 128:(k + 1) * 128, cc * CW:(cc + 1) * CW])
                xt_sb[k, cc] = t

        # persistent per-row-block stat columns
        negm_all = acc_pool.tile([128, NRB], DT.float32)
        ssum_all = acc_pool.tile([128, NRB], DT.float32)
        gsel_all = acc_pool.tile([128, NRB], DT.float32)

        for rb in range(NRB):
            rbs = slice(rb * 128, (rb + 1) * 128)

            # G = X_rows . S^T  -> [128, 100] psum
            gps = g_pool.tile([128, NL], DT.float32, tag="gps")
            for k in range(NK):
                nc.tensor.matmul(gps[:], xt_sb[k, 0][:, rbs], st_sb[k][:],
                                 start=(k == 0), stop=(k == NK - 1))

            rm = st_pool.tile([128, NCC], DT.float32, tag="rm")
            zrb = z_pool.tile([128, N], DT.float16, tag="z")

            for cc in range(NCC):
                ps = ps_pool.tile([128, CW], DT.float32, tag="ps")
                for h in range(2):
                    for k in range(NK):
                        nc.tensor.matmul(
                            ps[:, h * 512:(h + 1) * 512],
                            xt_sb[k, 0][:, rbs],
                            xt_sb[k, cc][:, h * 512:(h + 1) * 512],
                            start=(k == 0), stop=(k == NK - 1))
                if cc == 0:
                    off = rb * 128
                    nc.vector.scalar_tensor_tensor(
                        out=ps[:, off:off + 128], in0=ps[:, off:off + 128],
                        scalar=0.0, in1=ome_sb[:],
                        op0=ALU.bypass, op1=ALU.mult)
                # fused PSUM->SBUF(fp16) copy + per-chunk row max
                nc.vector.tensor_scalar(
                    out=zrb[:, cc * CW:(cc + 1) * CW], in0=ps[:],
                    scalar1=0.0, scalar2=-3.0e38,
                    op0=ALU.add, op1=ALU.max, accum_out=rm[:, cc:cc + 1])

            mfin = st_pool.tile([128, 1], DT.float32, tag="mfin")
            nc.vector.tensor_reduce(mfin[:], rm[:], axis=mybir.AxisListType.X,
                                    op=ALU.max)
            nc.vector.tensor_scalar_mul(negm_all[:, rb:rb + 1], mfin[:], -10.0)

            # gsel = G[i, label_i] via one-hot dot (accum_out is a sum)
            gscr = st_pool.tile([128, NL], DT.float32, tag="gscr")
            nc.vector.scalar_tensor_tensor(
                out=gscr[:], in0=kidx_sb[:], scalar=lrow_sb[:, rb:rb + 1],
                in1=gps[:], op0=ALU.is_equal, op1=ALU.mult,
                accum_out=gsel_all[:, rb:rb + 1])

            # sum_j exp(10*(z_ij - m_i)) in one big-FD pass
            escr = e_pool.tile([128, N], DT.float16, tag="escr")
            nc.scalar.activation(
                out=escr[:], in_=zrb[:], func=ACTF.Exp,
                bias=negm_all[:, rb:rb + 1], scale=10.0,
                accum_out=ssum_all[:, rb:rb + 1])

        # tail: all Ln at once (one act-table set switch), then final math
        lns_all = acc_pool.tile([128, NRB], DT.float32)
        nc.scalar.activation(lns_all[:], ssum_all[:], ACTF.Ln)
        t1 = acc_pool.tile([128, NRB], DT.float32)
        nc.vector.tensor_sub(t1[:], gsel_all[:], nrm_sb[:])
        t2 = acc_pool.tile([128, NRB], DT.float32)
        nc.vector.tensor_mul(t2[:], t1[:], icnt_sb[:])
        t3 = acc_pool.tile([128, NRB], DT.float32)
        nc.vector.tensor_sub(t3[:], t2[:], lns_all[:])
        t4 = acc_pool.tile([128, NRB], DT.float32)
        nc.vector.tensor_add(t4[:], t3[:], negm_all[:])
        res_sb = acc_pool.tile([128, NRB], DT.float32)
        nc.vector.tensor_mul(res_sb[:], t4[:], negw_sb[:])
        nc.sync.dma_start(res_d[:], res_sb[:])

    nc.compile()
    _NC_CACHE["nc"] = nc
    return nc


def _reset_device():
    try:
        import ctypes, jax
        jax.devices()
        ctypes.CDLL("/opt/axon/libaxon_pjrt.so").axon_reset()
    except Exception:
        pass


def _make_in_maps(features, labels, weights):
    f = np.ascontiguousarray(np.asarray(features, dtype=np.float32))
    lab = np.asarray(labels).astype(np.int32)
    w = np.asarray(weights, dtype=np.float32)

    xt16 = f.T.astype(np.float16)                       # [D, N]
    hist = np.bincount(lab, minlength=NL).astype(np.float64)
    icnt10_full = (10.0 / (hist[lab] - 1.0)).astype(np.float32)   # [N]
    nrm_full = (f.astype(np.float64) ** 2).sum(axis=1).astype(np.float32)

    S = np.zeros((NL, D), dtype=np.float64)
    np.add.at(S, lab, f.astype(np.float64))
    st16 = np.ascontiguousarray(S.T.astype(np.float16))  # [D, 100]

    kidx = np.broadcast_to(np.arange(NL, dtype=np.float32)[None, :], (128, NL))
    ome = (1.0 - np.eye(128)).astype(np.float32)

    in_maps = []
    for c in range(C):
        sl = slice(c * R, (c + 1) * R)
        perm = np.concatenate([
            np.arange(c * R, (c + 1) * R),
            np.arange(0, c * R),
            np.arange((c + 1) * R, N),
        ])
        in_maps.append({
            "xt": np.ascontiguousarray(xt16[:, perm]),
            "st": st16,
            "labrow": np.ascontiguousarray(
                lab[sl].astype(np.float32).reshape(NRB, 128).T),
            "negw": np.ascontiguousarray((-w[sl]).reshape(NRB, 128).T),
            "icnt10": np.ascontiguousarray(
                icnt10_full[sl].reshape(NRB, 128).T),
            "nrm": np.ascontiguousarray(nrm_full[sl].reshape(NRB, 128).T),
            "kidx": np.ascontiguousarray(kidx),
            "ome": ome,
        })

    return in_maps


def kernel(features, labels, weights):
    from concourse.bass_utils import run_bass_kernel_spmd

    w = np.asarray(weights, dtype=np.float32)
    nc = _build_nc()
    _reset_device()
    in_maps = _make_in_maps(features, labels, weights)
    out = run_bass_kernel_spmd(nc, in_maps, list(range(C)))
    total = np.float64(0.0)
    for c in range(C):
        total += out.results[c]["res"].astype(np.float64).sum()
    loss = total / np.float64(w.astype(np.float64).sum())
    return np.asarray(loss, dtype=np.float32)


# revision 3
# speedup vs baseline: 1.7657x; 1.1506x over previous
"""Supervised-contrastive loss (nn_ConLoss) on 8 Trainium2 NeuronCores.

Row-sharded data-parallel: each core computes its [1024, 8192] block of
Z = X X^T with fp8-e4m3 DoubleRow matmuls (2 MACs/cell/cycle), evacuates
PSUM split across the vector engine (fused copy+row-max) and the scalar
engine (plain copies, re-maxed by DVE at 4x from fp16 SBUF), then one
big-FD exp pass per row block on the scalar engine with sum-accumulate.
Positive-pair sums come algebraically from S = per-class feature sums:
sum_{j:lab_j=lab_i} z_ij = x_i . S_{lab_i} - ||x_i||^2 (tiny matmul +
one-hot dot), so no O(N^2) mask pass exists. Ln and the final per-row
loss math are batched once at the end. Host sums partial losses.
"""
import numpy as np

TEMPERATURE = 0.1
N, D, C = 8192, 512, 8
R = N // C            # 1024 rows per core
NRB = R // 128        # 8 row blocks of 128
CW = 1024             # elementwise tile width (2 psum banks)
NCC = N // CW         # 8 column chunks
NK = D // 128         # 4 k-subtiles
NKP = NK // 2         # 2 DoubleRow k-pairs
NL = 100              # num classes
ACT_CHUNKS = (1, 2)   # chunks evacuated by the scalar engine per row block

_NC_CACHE = {}


def _build_nc():
    if "nc" in _NC_CACHE:
        return _NC_CACHE["nc"]
    import concourse.tile as tile
    from concourse import bacc, mybir
    from contextlib import ExitStack

    DT = mybir.dt
    ALU = mybir.AluOpType
    ACTF = mybir.ActivationFunctionType
    DR = mybir.MatmulPerfMode.DoubleRow

    nc = bacc.Bacc("TRN2", target_bir_lowering=False, debug=False)
    xt_d = nc.dram_tensor("xt8", [128, NK, N], DT.float8e4, kind="ExternalInput")
    st_d = nc.dram_tensor("st8", [D, NL], DT.float8e4, kind="ExternalInput")
    lrow_d = nc.dram_tensor("labrow", [128, NRB], DT.float32, kind="ExternalInput")
    negw_d = nc.dram_tensor("negw", [128, NRB], DT.float32, kind="ExternalInput")
    icnt_d = nc.dram_tensor("icnt10", [128, NRB], DT.float32, kind="ExternalInput")
    nrm_d = nc.dram_tensor("nrm", [128, NRB], DT.float32, kind="ExternalInput")
    kidx_d = nc.dram_tensor("kidx", [128, NL], DT.float32, kind="ExternalInput")
    ome_d = nc.dram_tensor("ome", [128, 128], DT.float32, kind="ExternalInput")
    res_d = nc.dram_tensor("res", [128, NRB], DT.float32, kind="ExternalOutput")

    with tile.TileContext(nc) as tc, ExitStack() as ctx:
        xt_pool = ctx.enter_context(tc.tile_pool(name="xt", bufs=1))
        sml_pool = ctx.enter_context(tc.tile_pool(name="sml", bufs=1))
        z_pool = ctx.enter_context(tc.tile_pool(name="z", bufs=3))
        e_pool = ctx.enter_context(tc.tile_pool(name="e", bufs=2))
        dum_pool = ctx.enter_context(tc.tile_pool(name="dum", bufs=2))
        ps_pool = ctx.enter_context(tc.tile_pool(name="ps", bufs=3, space="PSUM"))
        g_pool = ctx.enter_context(tc.tile_pool(name="g", bufs=2, space="PSUM"))
        st_pool = ctx.enter_context(tc.tile_pool(name="st", bufs=2))
        acc_pool = ctx.enter_context(tc.tile_pool(name="acc", bufs=1))

        st_sb = []
        for k in range(NK):
            t = sml_pool.tile([128, NL], DT.float8e4, tag=f"st_{k}")
            nc.sync.dma_start(t[:], st_d[k * 128:(k + 1) * 128, :])
            st_sb.append(t)
        lrow_sb = sml_pool.tile([128, NRB], DT.float32)
        nc.sync.dma_start(lrow_sb[:], lrow_d[:])
        negw_sb = sml_pool.tile([128, NRB], DT.float32)
        nc.sync.dma_start(negw_sb[:], negw_d[:])
        icnt_sb = sml_pool.tile([128, NRB], DT.float32)
        nc.sync.dma_start(icnt_sb[:], icnt_d[:])
        nrm_sb = sml_pool.tile([128, NRB], DT.float32)
        nc.sync.dma_start(nrm_sb[:], nrm_d[:])
        kidx_sb = sml_pool.tile([128, NL], DT.float32)
        nc.sync.dma_start(kidx_sb[:], kidx_d[:])
        ome_sb = sml_pool.tile([128, 128], DT.float32)
        nc.sync.dma_start(ome_sb[:], ome_d[:])

        # fp8 xt tiles [128, 2, CW] per (k-pair, chunk); chunk-major order
        xt_sb = {}
        for cc in range(NCC):
            for kp in range(NKP):
                t = xt_pool.tile([128, 2, CW], DT.float8e4, tag=f"xt_{kp}_{cc}")
                nc.sync.dma_start(
                    t[:], xt_d[:, 2 * kp:2 * kp + 2, cc * CW:(cc + 1) * CW])
                xt_sb[kp, cc] = t

        negm_all = acc_pool.tile([128, NRB], DT.float32)
        ssum_all = acc_pool.tile([128, NRB], DT.float32)
        gsel_all = acc_pool.tile([128, NRB], DT.float32)

        z_tiles = []
        for rb in range(NRB):
            rbs = slice(rb * 128, (rb + 1) * 128)

            # G = X_rows . S^T -> [128, 100] psum (normal fp8 matmuls)
            gps = g_pool.tile([128, NL], DT.float32, tag="gps")
            for k in range(NK):
                nc.tensor.matmul(gps[:], xt_sb[k // 2, 0][:, k % 2, rbs],
                                 st_sb[k][:], start=(k == 0), stop=(k == NK - 1))

            rm = st_pool.tile([128, NCC], DT.float32, tag="rm")
            zrb = z_pool.tile([128, N], DT.float16, tag="z")
            z_tiles.append(zrb)

            for cc in range(NCC):
                ps = ps_pool.tile([128, CW], DT.float32, tag="ps")
                for h in range(2):
                    for kp in range(NKP):
                        nc.tensor.matmul(
                            ps[:, h * 512:(h + 1) * 512],
                            xt_sb[kp, 0][:, :, rbs],
                            xt_sb[kp, cc][:, :, h * 512:(h + 1) * 512],
                            start=(kp == 0), stop=(kp == NKP - 1),
                            perf_mode=DR)
                if cc == 0:
                    off = rb * 128
                    nc.vector.scalar_tensor_tensor(
                        out=ps[:, off:off + 128], in0=ps[:, off:off + 128],
                        scalar=0.0, in1=ome_sb[:],
                        op0=ALU.bypass, op1=ALU.mult)
                if cc in ACT_CHUNKS:
                    # scalar engine evacuates; DVE re-maxes at 4x later
                    nc.scalar.copy(zrb[:, cc * CW:(cc + 1) * CW], ps[:])
                else:
                    nc.vector.tensor_scalar(
                        out=zrb[:, cc * CW:(cc + 1) * CW], in0=ps[:],
                        scalar1=0.0, scalar2=-3.0e38,
                        op0=ALU.add, op1=ALU.max, accum_out=rm[:, cc:cc + 1])

            for cc in ACT_CHUNKS:
                dscr = dum_pool.tile([128, CW], DT.float16, tag="dscr")
                nc.vector.tensor_scalar(
                    out=dscr[:], in0=zrb[:, cc * CW:(cc + 1) * CW],
                    scalar1=0.0, scalar2=-3.0e38,
                    op0=ALU.add, op1=ALU.max, accum_out=rm[:, cc:cc + 1])

            mfin = st_pool.tile([128, 1], DT.float32, tag="mfin")
            nc.vector.tensor_reduce(mfin[:], rm[:], axis=mybir.AxisListType.X,
                                    op=ALU.max)
            nc.vector.tensor_scalar_mul(negm_all[:, rb:rb + 1], mfin[:], -10.0)

            gscr = st_pool.tile([128, NL], DT.float32, tag="gscr")
            nc.vector.scalar_tensor_tensor(
                out=gscr[:], in0=kidx_sb[:], scalar=lrow_sb[:, rb:rb + 1],
                in1=gps[:], op0=ALU.is_equal, op1=ALU.mult,
                accum_out=gsel_all[:, rb:rb + 1])

            # exp of the PREVIOUS row block (keeps ACT copies of this rb
            # ahead of it in the ACT queue -> no cross-engine serial chain)
            if rb > 0:
                pe = rb - 1
                escr = e_pool.tile([128, N], DT.float16, tag="escr")
                nc.scalar.activation(
                    out=escr[:], in_=z_tiles[pe][:], func=ACTF.Exp,
                    bias=negm_all[:, pe:pe + 1], scale=10.0,
                    accum_out=ssum_all[:, pe:pe + 1])

        pe = NRB - 1
        escr = e_pool.tile([128, N], DT.float16, tag="escr")
        nc.scalar.activation(
            out=escr[:], in_=z_tiles[pe][:], func=ACTF.Exp,
            bias=negm_all[:, pe:pe + 1], scale=10.0,
            accum_out=ssum_all[:, pe:pe + 1])

        lns_all = acc_pool.tile([128, NRB], DT.float32)
        nc.scalar.activation(lns_all[:], ssum_all[:], ACTF.Ln)
        t1 = acc_pool.tile([128, NRB], DT.float32)
        nc.vector.tensor_sub(t1[:], gsel_all[:], nrm_sb[:])
        t2 = acc_pool.tile([128, NRB], DT.float32)
        nc.vector.tensor_mul(t2[:], t1[:], icnt_sb[:])
        t3 = acc_pool.tile([128, NRB], DT.float32)
        nc.vector.tensor_sub(t3[:], t2[:], lns_all[:])
        t4 = acc_pool.tile([128, NRB], DT.float32)
        nc.vector.tensor_add(t4[:], t3[:], negm_all[:])
        res_sb = acc_pool.tile([128, NRB], DT.float32)
        nc.vector.tensor_mul(res_sb[:], t4[:], negw_sb[:])
        nc.sync.dma_start(res_d[:], res_sb[:])

    nc.compile()
    _NC_CACHE["nc"] = nc
    return nc


def _reset_device():
    try:
        import ctypes, jax
        jax.devices()
        ctypes.CDLL("/opt/axon/libaxon_pjrt.so").axon_reset()
    except Exception:
        pass


def _make_in_maps(features, labels, weights):
    from concourse import mybir
    f8dt = mybir.dt.np(mybir.dt.float8e4)

    f = np.ascontiguousarray(np.asarray(features, dtype=np.float32))
    lab = np.asarray(labels).astype(np.int32)
    w = np.asarray(weights, dtype=np.float32)

    xt = f.T.astype(np.float32)                          # [D, N]
    hist = np.bincount(lab, minlength=NL).astype(np.float64)
    icnt10_full = (10.0 / (hist[lab] - 1.0)).astype(np.float32)
    nrm_full = (f.astype(np.float64) ** 2).sum(axis=1).astype(np.float32)

    S = np.zeros((NL, D), dtype=np.float64)
    np.add.at(S, lab, f.astype(np.float64))
    st8 = np.ascontiguousarray(S.T.astype(np.float32).astype(f8dt))

    kidx = np.broadcast_to(np.arange(NL, dtype=np.float32)[None, :], (128, NL))
    ome = (1.0 - np.eye(128)).astype(np.float32)

    in_maps = []
    for c in range(C):
        sl = slice(c * R, (c + 1) * R)
        perm = np.concatenate([
            np.arange(c * R, (c + 1) * R),
            np.arange(0, c * R),
            np.arange((c + 1) * R, N),
        ])
        xtp8 = xt[:, perm].astype(f8dt)                  # [D, N] fp8
        # [128, NK, N]: element [p, ks, n] = xtp8[ks*128+p, n]
        xt8 = np.ascontiguousarray(
            xtp8.reshape(NK, 128, N).transpose(1, 0, 2))
        in_maps.append({
            "xt8": xt8,
            "st8": st8,
            "labrow": np.ascontiguousarray(
                lab[sl].astype(np.float32).reshape(NRB, 128).T),
            "negw": np.ascontiguousarray((-w[sl]).reshape(NRB, 128).T),
            "icnt10": np.ascontiguousarray(
                icnt10_full[sl].reshape(NRB, 128).T),
            "nrm": np.ascontiguousarray(nrm_full[sl].reshape(NRB, 128).T),
            "kidx": np.ascontiguousarray(kidx),
            "ome": ome,
        })

    return in_maps


def kernel(features, labels, weights):
    from concourse.bass_utils import run_bass_kernel_spmd

    w = np.asarray(weights, dtype=np.float32)
    nc = _build_nc()
    _reset_device()
    in_maps = _make_in_maps(features, labels, weights)
    out = run_bass_kernel_spmd(nc, in_maps, list(range(C)))
    total = np.float64(0.0)
    for c in range(C):
        total += out.results[c]["res"].astype(np.float64).sum()
    loss = total / np.float64(w.astype(np.float64).sum())
    return np.asarray(loss, dtype=np.float32)


# revision 4
# speedup vs baseline: 2.1225x; 1.2021x over previous
"""Supervised-contrastive loss (nn_ConLoss) on 8 Trainium2 NeuronCores.

Row-sharded data-parallel: each core computes its [1024, 8192] block of
Z = X X^T with fp8-e4m3 DoubleRow matmuls (2 MACs/cell/cycle), evacuates
PSUM split across the vector engine (fused copy+row-max) and the scalar
engine (plain copies, re-maxed by DVE at 4x from fp16 SBUF), then one
big-FD exp pass per row block on the scalar engine with sum-accumulate.
Positive-pair sums come algebraically from S = per-class feature sums:
sum_{j:lab_j=lab_i} z_ij = x_i . S_{lab_i} - ||x_i||^2 (tiny matmul +
one-hot dot), so no O(N^2) mask pass exists. Ln and the final per-row
loss math are batched once at the end. Host sums partial losses.
"""
import numpy as np

TEMPERATURE = 0.1
N, D, C = 8192, 512, 8
R = N // C            # 1024 rows per core
NRB = R // 128        # 8 row blocks of 128
CW = 1024             # elementwise tile width (2 psum banks)
NCC = N // CW         # 8 column chunks
NK = D // 128         # 4 k-subtiles
NKP = NK // 2         # 2 DoubleRow k-pairs
NL = 100              # num classes

_NC_CACHE = {}


def _build_nc():
    if "nc" in _NC_CACHE:
        return _NC_CACHE["nc"]
    import concourse.tile as tile
    from concourse import bacc, mybir
    from contextlib import ExitStack

    DT = mybir.dt
    ALU = mybir.AluOpType
    ACTF = mybir.ActivationFunctionType
    DR = mybir.MatmulPerfMode.DoubleRow

    nc = bacc.Bacc("TRN2", target_bir_lowering=False, debug=False)
    xt_d = nc.dram_tensor("xt8", [128, NK, N], DT.float8e4, kind="ExternalInput")
    st_d = nc.dram_tensor("st8", [D, NL], DT.float8e4, kind="ExternalInput")
    lrow_d = nc.dram_tensor("labrow", [128, NRB], DT.float32, kind="ExternalInput")
    negw_d = nc.dram_tensor("negw", [128, NRB], DT.float32, kind="ExternalInput")
    icnt_d = nc.dram_tensor("icnt10", [128, NRB], DT.float32, kind="ExternalInput")
    nrm_d = nc.dram_tensor("nrm", [128, NRB], DT.float32, kind="ExternalInput")
    kidx_d = nc.dram_tensor("kidx", [128, NL], DT.float32, kind="ExternalInput")
    ome_d = nc.dram_tensor("ome", [128, 128], DT.float32, kind="ExternalInput")
    res_d = nc.dram_tensor("res", [128, NRB], DT.float32, kind="ExternalOutput")

    with tile.TileContext(nc) as tc, ExitStack() as ctx:
        xt_pool = ctx.enter_context(tc.tile_pool(name="xt", bufs=1))
        sml_pool = ctx.enter_context(tc.tile_pool(name="sml", bufs=1))
        z_pool = ctx.enter_context(tc.tile_pool(name="z", bufs=2))
        e_pool = ctx.enter_context(tc.tile_pool(name="e", bufs=2))
        ps_pool = ctx.enter_context(tc.tile_pool(name="ps", bufs=3, space="PSUM"))
        g_pool = ctx.enter_context(tc.tile_pool(name="g", bufs=2, space="PSUM"))
        st_pool = ctx.enter_context(tc.tile_pool(name="st", bufs=2))
        acc_pool = ctx.enter_context(tc.tile_pool(name="acc", bufs=1))

        st_sb = []
        for k in range(NK):
            t = sml_pool.tile([128, NL], DT.float8e4, tag=f"st_{k}")
            nc.sync.dma_start(t[:], st_d[k * 128:(k + 1) * 128, :])
            st_sb.append(t)
        lrow_sb = sml_pool.tile([128, NRB], DT.float32)
        nc.sync.dma_start(lrow_sb[:], lrow_d[:])
        negw_sb = sml_pool.tile([128, NRB], DT.float32)
        nc.sync.dma_start(negw_sb[:], negw_d[:])
        icnt_sb = sml_pool.tile([128, NRB], DT.float32)
        nc.sync.dma_start(icnt_sb[:], icnt_d[:])
        nrm_sb = sml_pool.tile([128, NRB], DT.float32)
        nc.sync.dma_start(nrm_sb[:], nrm_d[:])
        kidx_sb = sml_pool.tile([128, NL], DT.float32)
        nc.sync.dma_start(kidx_sb[:], kidx_d[:])
        ome_sb = sml_pool.tile([128, 128], DT.float32)
        nc.sync.dma_start(ome_sb[:], ome_d[:])

        # fp8 xt tiles [128, 2, CW] per (k-pair, chunk); chunk-major order
        xt_sb = {}
        for cc in range(NCC):
            for kp in range(NKP):
                t = xt_pool.tile([128, 2, CW], DT.float8e4, tag=f"xt_{kp}_{cc}")
                nc.sync.dma_start(
                    t[:], xt_d[:, 2 * kp:2 * kp + 2, cc * CW:(cc + 1) * CW])
                xt_sb[kp, cc] = t

        negm_all = acc_pool.tile([128, NRB], DT.float32)
        ssum_all = acc_pool.tile([128, NRB], DT.float32)
        gsel_all = acc_pool.tile([128, NRB], DT.float32)

        z_tiles = []
        for rb in range(NRB):
            rbs = slice(rb * 128, (rb + 1) * 128)

            # G = X_rows . S^T -> [128, 100] psum (normal fp8 matmuls)
            gps = g_pool.tile([128, NL], DT.float32, tag="gps")
            for k in range(NK):
                nc.tensor.matmul(gps[:], xt_sb[k // 2, 0][:, k % 2, rbs],
                                 st_sb[k][:], start=(k == 0), stop=(k == NK - 1))

            rm = st_pool.tile([128, NCC], DT.float32, tag="rm")
            zrb = z_pool.tile([128, N], DT.float16, tag="z")
            z_tiles.append(zrb)

            for cc in range(NCC):
                ps = ps_pool.tile([128, CW], DT.float32, tag="ps")
                for h in range(2):
                    for kp in range(NKP):
                        nc.tensor.matmul(
                            ps[:, h * 512:(h + 1) * 512],
                            xt_sb[kp, 0][:, :, rbs],
                            xt_sb[kp, cc][:, :, h * 512:(h + 1) * 512],
                            start=(kp == 0), stop=(kp == NKP - 1),
                            perf_mode=DR)
                if cc == 0:
                    off = rb * 128
                    nc.vector.scalar_tensor_tensor(
                        out=ps[:, off:off + 128], in0=ps[:, off:off + 128],
                        scalar=0.0, in1=ome_sb[:],
                        op0=ALU.bypass, op1=ALU.mult)
                nc.vector.tensor_scalar(
                    out=zrb[:, cc * CW:(cc + 1) * CW], in0=ps[:],
                    scalar1=0.0, scalar2=-3.0e38,
                    op0=ALU.add, op1=ALU.max, accum_out=rm[:, cc:cc + 1])

            mfin = st_pool.tile([128, 1], DT.float32, tag="mfin")
            nc.vector.tensor_reduce(mfin[:], rm[:], axis=mybir.AxisListType.X,
                                    op=ALU.max)
            nc.vector.tensor_scalar_mul(negm_all[:, rb:rb + 1], mfin[:], -10.0)

            gscr = st_pool.tile([128, NL], DT.float32, tag="gscr")
            nc.vector.scalar_tensor_tensor(
                out=gscr[:], in0=kidx_sb[:], scalar=lrow_sb[:, rb:rb + 1],
                in1=gps[:], op0=ALU.is_equal, op1=ALU.mult,
                accum_out=gsel_all[:, rb:rb + 1])

            escr = e_pool.tile([128, N], DT.float16, tag="escr")
            nc.scalar.activation(
                out=escr[:], in_=zrb[:], func=ACTF.Exp,
                bias=negm_all[:, rb:rb + 1], scale=10.0,
                accum_out=ssum_all[:, rb:rb + 1])

        lns_all = acc_pool.tile([128, NRB], DT.float32)
        nc.scalar.activation(lns_all[:], ssum_all[:], ACTF.Ln)
        t1 = acc_pool.tile([128, NRB], DT.float32)
        nc.vector.tensor_sub(t1[:], gsel_all[:], nrm_sb[:])
        t2 = acc_pool.tile([128, NRB], DT.float32)
        nc.vector.tensor_mul(t2[:], t1[:], icnt_sb[:])
        t3 = acc_pool.tile([128, NRB], DT.float32)
        nc.vector.tensor_sub(t3[:], t2[:], lns_all[:])
        t4 = acc_pool.tile([128, NRB], DT.float32)
        nc.vector.tensor_add(t4[:], t3[:], negm_all[:])
        res_sb = acc_pool.tile([128, NRB], DT.float32)
        nc.vector.tensor_mul(res_sb[:], t4[:], negw_sb[:])
        nc.sync.dma_start(res_d[:], res_sb[:])

    nc.compile()
    _NC_CACHE["nc"] = nc
    return nc


def _reset_device():
    try:
        import ctypes, jax
        jax.devices()
        ctypes.CDLL("/opt/axon/libaxon_pjrt.so").axon_reset()
    except Exception:
        pass


def _make_in_maps(features, labels, weights):
    from concourse import mybir
    f8dt = mybir.dt.np(mybir.dt.float8e4)

    f = np.ascontiguousarray(np.asarray(features, dtype=np.float32))
    lab = np.asarray(labels).astype(np.int32)
    w = np.asarray(weights, dtype=np.float32)

    xt = f.T.astype(np.float32)                          # [D, N]
    hist = np.bincount(lab, minlength=NL).astype(np.float64)
    icnt10_full = (10.0 / (hist[lab] - 1.0)).astype(np.float32)
    nrm_full = (f.astype(np.float64) ** 2).sum(axis=1).astype(np.float32)

    S = np.zeros((NL, D), dtype=np.float64)
    np.add.at(S, lab, f.astype(np.float64))
    st8 = np.ascontiguousarray(S.T.astype(np.float32).astype(f8dt))

    kidx = np.broadcast_to(np.arange(NL, dtype=np.float32)[None, :], (128, NL))
    ome = (1.0 - np.eye(128)).astype(np.float32)

    in_maps = []
    for c in range(C):
        sl = slice(c * R, (c + 1) * R)
        perm = np.concatenate([
            np.arange(c * R, (c + 1) * R),
            np.arange(0, c * R),
            np.arange((c + 1) * R, N),
        ])
        xtp8 = xt[:, perm].astype(f8dt)                  # [D, N] fp8
        # [128, NK, N]: element [p, ks, n] = xtp8[ks*128+p, n]
        xt8 = np.ascontiguousarray(
            xtp8.reshape(NK, 128, N).transpose(1, 0, 2))
        in_maps.append({
            "xt8": xt8,
            "st8": st8,
            "labrow": np.ascontiguousarray(
                lab[sl].astype(np.float32).reshape(NRB, 128).T),
            "negw": np.ascontiguousarray((-w[sl]).reshape(NRB, 128).T),
            "icnt10": np.ascontiguousarray(
                icnt10_full[sl].reshape(NRB, 128).T),
            "nrm": np.ascontiguousarray(nrm_full[sl].reshape(NRB, 128).T),
            "kidx": np.ascontiguousarray(kidx),
            "ome": ome,
        })

    return in_maps


def kernel(features, labels, weights):
    from concourse.bass_utils import run_bass_kernel_spmd

    w = np.asarray(weights, dtype=np.float32)
    nc = _build_nc()
    _reset_device()
    in_maps = _make_in_maps(features, labels, weights)
    out = run_bass_kernel_spmd(nc, in_maps, list(range(C)))
    total = np.float64(0.0)
    for c in range(C):
        total += out.results[c]["res"].astype(np.float64).sum()
    loss = total / np.float64(w.astype(np.float64).sum())
    return np.asarray(loss, dtype=np.float32)
